# revision 20
# baseline (speedup 1.0000x reference)
"""L2 (spectral) contrastive loss on 8 Trainium2 NeuronCores.

Math: with G_x = x.T @ x and G_y = y.T @ y (both [D, D]),
    sum_{i,j} <x_i, y_j>^2 = ||x @ y.T||_F^2 = tr(G_x @ G_y) = sum(G_x * G_y)
so the loss needs only the two Gram matrices (2*N*D^2 MACs) instead of the
[N, N] pairwise product (N^2*D MACs) - a 5.3x FLOP reduction at N=8192, D=768.

Sharding: rows of x and y are split across the 8 cores. Each core computes
partial Grams over its 1024 rows (bf16 matmuls, fp32 PSUM accumulation,
upper-triangle tiles only - Grams are symmetric; strict-upper entries are
pre-scaled by 2 on the G_y side so the reduced dot needs no triangle
bookkeeping), plus the diagonal terms z_i = <x_i, y_i> (fp32).

Cross-core reduction: a single fp16 ReduceScatter of the packed
partials gives every core 1/8 of the summed Grams (fp8 was tried and
reverted: the collective is element-rate-bound in the CCE, so halving
the bytes does not speed it up, and e4m3's 240 max overflows); the
pack layout interleaves
G_x and G_y in 336-column blocks so each core's shard reshapes to
[128, 672] with matching G_x|G_y halves per partition - the local dot is
one flat scalar_tensor_tensor. A tiny [1,3] fp32 AllGather then combines
the 8 partial dots plus the z-sums, and every core computes the final
loss (core 0's output is returned).

Timing note: measured on this runtime, the first collective starts at
max(rendezvous-barrier end, trigger) + ~11.4us, where the barrier ends
21.5us + 23-47us (launch-skew lottery) into the run. The compute phase
(~36-40us trigger) largely hides under the barrier, so exec time is
dominated by barrier luck + ReduceScatter (~18-23us, latency-bound:
fp8/int32-packed wires with fewer bytes/elements measured no faster)
+ the epilogue chain. Engine FIFO emission order and rates matter:
ACT casts/copies measure ~2.3x slower than DVE (~1.8 vs 0.8 ns/col),
so all input casts ride DVE (arrival-paced) and the PSUM pack copies
are rate-balanced across DVE/ACT, emitted before the remaining y-casts
so nothing head-of-line blocks the copies that release PSUM for the
G_y burst (doorbell ~43us vs ~47us before this balancing).
"""
import numpy as np
from contextlib import ExitStack

from concourse import bacc, tile, mybir
from concourse.bass_utils import run_bass_kernel_spmd

N_CORES = 8
N, D = 8192, 768
ROWS = N // N_CORES          # 1024 rows per core
P = 128                      # SBUF partitions
KCH = ROWS // P              # 8 contraction chunks per core
MS = D // P                  # 6 output slabs per Gram

# upper-triangle slab widths and packed column offsets
WIDTHS = [D - P * m for m in range(MS)]              # [768,640,512,384,256,128]
COFF = [sum(WIDTHS[:m]) for m in range(MS)]          # prefix offsets
GCOLS = sum(WIDTHS)                                  # 2688 per Gram
PACK_COLS = 2 * GCOLS                                # 5376 (no pad: 128*5376 % 8 == 0)
RUN = GCOLS // 8                                     # 336: shard-block run length
# PSUM slab padding to full banks (512 fp32 = 2 KiB) so every matmul chunk
# stays inside one bank: [1024,1024,512,512,512,512] = 8 banks exactly.
PADW = [((w + 511) // 512) * 512 for w in WIDTHS]

JUNK_MMS = 34                # PE warm-up stream (~3.5us cold) before real MMs

F32 = mybir.dt.float32
F16 = mybir.dt.float16
F8 = mybir.dt.float8e4
BF16 = mybir.dt.bfloat16
U8 = mybir.dt.uint8
WIRE_SCALE = 1.0             # fp16 wire needs no range scaling

_CACHE = {}


def _free_chunks(width):
    """Split [0, width) at the 512-column PSUM bank boundary."""
    if width <= 512:
        return [(0, width)]
    return [(0, 512), (512, width)]


def _build():
    nc = bacc.Bacc("TRN2", target_bir_lowering=False, debug=False,
                   num_devices=N_CORES)
    x_ap = nc.dram_tensor("x", [ROWS, D], F32, kind="ExternalInput").ap()
    y_ap = nc.dram_tensor("y", [ROWS, D], F32, kind="ExternalInput").ap()
    loss_ap = nc.dram_tensor("loss", [1, 1], F32, kind="ExternalOutput").ap()

    inv_nn1 = 1.0 / (float(N) * (N - 1))
    rg = [list(range(N_CORES))]

    with tile.TileContext(nc) as tc:
        with ExitStack() as ctx:
            sb = ctx.enter_context(tc.tile_pool(name="sb", bufs=1))
            ps = ctx.enter_context(tc.tile_pool(name="ps", bufs=1, space="PSUM"))
            dram = ctx.enter_context(tc.tile_pool(name="dram", bufs=1, space="DRAM"))

            # ---- PSUM slabs (allocated up front; 8 banks total) ----
            slabs = []
            for m in range(MS):
                slab = ps.tile([P, WIDTHS[m]], F32, tag=f"slab{m}", bufs=1,
                               padded_shape=[P, PADW[m]], name=f"slab{m}")
                slabs.append(slab)

            # ---- PE warm-up: junk matmuls while inputs stream in ----
            junk_src = sb.tile([P, P], BF16)
            nc.gpsimd.memset(junk_src[:], 0.5)
            for _ in range(JUNK_MMS):
                nc.tensor.matmul(slabs[0][:, 0:P], junk_src[:], junk_src[:],
                                 start=True, stop=True)

            # ---- input loads: [1024, 768] -> [128p, 8k, 768], per k-chunk ----
            xt = sb.tile([P, KCH, D], F32)
            yt = sb.tile([P, KCH, D], F32)
            xr = x_ap.rearrange("(n p) d -> p n d", p=P)
            yr = y_ap.rearrange("(n p) d -> p n d", p=P)
            for k in range(KCH):
                nc.sync.dma_start(xt[:, k, :], xr[:, k, :])
            for k in range(KCH):
                nc.sync.dma_start(yt[:, k, :], yr[:, k, :])

            # ---- casts to bf16: even chunks on DVE, odd on ACT ----
            xb = sb.tile([P, KCH, D], BF16)
            yb = sb.tile([P, KCH, D], BF16)
            for k in range(KCH):
                nc.vector.tensor_copy(xb[:, k, :], xt[:, k, :])
            # y0 on ACT (idle until y arrives); y1-y7 go to DVE after the
            # G_x pack copies so nothing head-of-line blocks those copies.
            nc.scalar.copy(yb[:, 0, :], yt[:, 0, :])

            # ---- G_x: k-outer so matmuls chase DMA arrivals ----
            for k in range(KCH):
                for m in range(MS):
                    for (c0, c1) in _free_chunks(WIDTHS[m]):
                        nc.tensor.matmul(
                            slabs[m][:, c0:c1],
                            xb[:, k, P * m:P * (m + 1)],
                            xb[:, k, P * m + c0:P * m + c1],
                            start=(k == 0), stop=(k == KCH - 1),
                        )

            # ---- pack staging (fp16) + DRAM collective input ----
            # cin column layout: 8 blocks of 672 = [Gx run b (336) | Gy run b]
            # so each rank's ReduceScatter shard reshapes to [128, 672] with
            # aligned Gx/Gy halves per partition.
            cin = dram.tile([P, PACK_COLS], F16)
            cin_v = cin[:].rearrange("p (b t) -> p b t", b=8)   # [128, 8, 672]

            stx = sb.tile([P, GCOLS], F16)
            for m in range(MS):
                w = WIDTHS[m]
                if m < 3:
                    nc.vector.tensor_copy(stx[:, COFF[m]:COFF[m] + w],
                                          slabs[m][:, 0:w])
                else:
                    nc.scalar.copy(stx[:, COFF[m]:COFF[m] + w],
                                   slabs[m][:, 0:w])
            stx_v = stx[:].rearrange("p (b c) -> p b c", b=8)   # [128, 8, 336]
            nc.sync.dma_start(cin_v[:, :, 0:RUN], stx_v[:])

            for k in range(1, KCH):
                nc.vector.tensor_copy(yb[:, k, :], yt[:, k, :])

            # ---- G_y matmuls (slabs reused; WAR dep on the G_x copies) ----
            for k in range(KCH):
                for m in range(MS):
                    for (c0, c1) in _free_chunks(WIDTHS[m]):
                        nc.tensor.matmul(
                            slabs[m][:, c0:c1],
                            yb[:, k, P * m:P * (m + 1)],
                            yb[:, k, P * m + c0:P * m + c1],
                            start=(k == 0), stop=(k == KCH - 1),
                        )

            # ---- G_y pack: diag blocks x1, strict-upper x2 (fp16) ----
            sty = sb.tile([P, GCOLS], F16)
            for m in range(MS):
                w = WIDTHS[m]
                if m < 3:
                    nc.vector.tensor_copy(sty[:, COFF[m]:COFF[m] + P],
                                          slabs[m][:, 0:P])
                    if w > P:
                        nc.vector.tensor_scalar_mul(
                            sty[:, COFF[m] + P:COFF[m] + w], slabs[m][:, P:w],
                            2.0)
                else:
                    nc.scalar.copy(sty[:, COFF[m]:COFF[m] + P],
                                   slabs[m][:, 0:P])
                    if w > P:
                        nc.scalar.mul(sty[:, COFF[m] + P:COFF[m] + w],
                                      slabs[m][:, P:w], 2.0)
            sty_v = sty[:].rearrange("p (b c) -> p b c", b=8)
            # two halves so the first DMA overlaps the remaining copies
            # issue the two halves on the two HWDGE engines (Sync + ACT) so
            # the second descriptor-gen does not queue behind the first
            nc.sync.dma_start(cin_v[:, 0:4, RUN:2 * RUN], sty_v[:, 0:4, :])
            nc.scalar.dma_start(cin_v[:, 4:8, RUN:2 * RUN], sty_v[:, 4:8, :])

            # ---- diagonal terms z_i = <x_i, y_i> on DVE (accum per chunk) ----
            zscr = sb.tile([P, D], F32)
            zcols = sb.tile([P, KCH], F32)
            for k in range(KCH):
                nc.vector.scalar_tensor_tensor(
                    zscr[:], xb[:, k, :], 1.0, yb[:, k, :],
                    mybir.AluOpType.mult, mybir.AluOpType.mult,
                    accum_out=zcols[:, k:k + 1],
                )
            zsq = sb.tile([P, KCH], F32)
            nc.gpsimd.tensor_mul(zsq[:], zcols[:], zcols[:])
            zred = sb.tile([P, 2], F32)
            nc.vector.tensor_reduce(zred[:, 0:1], zcols[:], mybir.AxisListType.X,
                                    mybir.AluOpType.add)
            nc.vector.tensor_reduce(zred[:, 1:2], zsq[:], mybir.AxisListType.X,
                                    mybir.AluOpType.add)
            # agin = [dot_partial, sum(z), sum(z^2)]; z parts filled here
            agin = sb.tile([1, 3], F32)
            nc.gpsimd.tensor_reduce(agin[0:1, 1:3], zred[:], mybir.AxisListType.C,
                                    mybir.AluOpType.add)


            # ---- fp16 ReduceScatter: each rank gets 1/8 of the summed pack ----
            cout = dram.tile([P, PACK_COLS // 8], F16)
            nc.gpsimd.collective_compute(
                "ReduceScatter", mybir.AluOpType.add, replica_groups=rg,
                ins=[cin.opt()], outs=[cout.opt()],
            )

            # ---- local shard dot: [128, 672] with Gx|Gy halves aligned ----
            shard = sb.tile([P, PACK_COLS // 8], F16)
            nc.sync.dma_start(shard[:], cout[:])
            dscr = sb.tile([P, RUN], F16)  # fp16 out: fp8 product would clip
            dcol = sb.tile([P, 1], F32)
            nc.vector.scalar_tensor_tensor(
                dscr[:], shard[:, 0:RUN], 1.0, shard[:, RUN:2 * RUN],
                mybir.AluOpType.mult, mybir.AluOpType.mult,
                accum_out=dcol[:],
            )
            nc.gpsimd.tensor_reduce(agin[0:1, 0:1], dcol[:], mybir.AxisListType.C,
                                    mybir.AluOpType.add)

            # ---- tiny fp32 AllGather of [dot, sum(z), sum(z^2)] ----
            agin_d = dram.tile([1, 3], F32)
            agout_d = dram.tile([N_CORES, 3], F32, addr_space="Shared")
            nc.sync.dma_start(agin_d[:], agin[:])
            nc.gpsimd.collective_compute(
                "AllGather", mybir.AluOpType.bypass, replica_groups=rg,
                ins=[agin_d.opt()], outs=[agout_d.opt()],
            )
            agout = sb.tile([N_CORES, 3], F32)
            nc.sync.dma_start(agout[:], agout_d[:])
            tot = sb.tile([1, 3], F32)
            nc.gpsimd.tensor_reduce(tot[:], agout[:], mybir.AxisListType.C,
                                    mybir.AluOpType.add)

            # ---- finale: loss = inv*(dot - z2) - (2/N)*z1 ----
            u = sb.tile([1, 1], F32)
            unscale = 1.0 / (WIRE_SCALE * WIRE_SCALE)
            nc.vector.scalar_tensor_tensor(
                u[:], tot[0:1, 0:1], unscale, tot[0:1, 2:3],
                mybir.AluOpType.mult, mybir.AluOpType.subtract)
            r0 = sb.tile([1, 1], F32)
            nc.vector.tensor_scalar_mul(r0[:], u[:], inv_nn1)
            res = sb.tile([1, 1], F32)
            nc.vector.scalar_tensor_tensor(
                res[:], tot[0:1, 1:2], -2.0 / N, r0[:],
                mybir.AluOpType.mult, mybir.AluOpType.add,
            )
            nc.sync.dma_start(loss_ap[:], res[:])

    nc.compile()
    return nc


def _get_nc():
    if "nc" not in _CACHE:
        _CACHE["nc"] = _build()
    return _CACHE["nc"]


def _run(x, y, trace=False, **trace_kwargs):
    nc = _get_nc()
    x = np.ascontiguousarray(np.asarray(x, dtype=np.float32))
    y = np.ascontiguousarray(np.asarray(y, dtype=np.float32))
    assert x.shape == (N, D) and y.shape == (N, D)
    in_maps = [
        {"x": x[c * ROWS:(c + 1) * ROWS], "y": y[c * ROWS:(c + 1) * ROWS]}
        for c in range(N_CORES)
    ]
    res = run_bass_kernel_spmd(nc, in_maps, list(range(N_CORES)), trace=trace,
                               **trace_kwargs)
    loss = np.float32(res.results[0]["loss"][0, 0])
    return np.asarray(loss, dtype=np.float32).reshape(()), res


def kernel(x, y):
    out, _ = _run(x, y, trace=False)
    return out
